# revision 8
# baseline (speedup 1.0000x reference)
"""LIF (leaky integrate-and-fire) spiking recurrence on 8 Trainium2 cores.

Full input x: [T*bs, C, H, W] = [256, 128, 32, 32] f32 with T=8, bs=32.
Recurrence over T only, elementwise elsewhere:
    u_t = TAU * u_{t-1} * (1 - (u_{t-1} > VTH)) + x_t ;  o_t = (u_t > VTH)

Sharding: fully data-parallel over batch (bs=32 -> 4 per core), no
collectives. Each core's [4,128,32,32] per-timestep slab is a flat
[128 partitions, 4096] tile; the whole 16 MiB input stays SBUF-resident
and the state is computed in place over it.

Three structural tricks vs the naive f32 version (~97us):

1. Byte spikes: output is pure 0/1, so ACT computes s = sign(v - 2^t) as
   uint8 in one pass; host maps byte==1 -> 1.0f (exact under any
   float->u8 conversion semantics). Store traffic 16.78 -> 4.19 MB/core.

2. Scaled state: with v_t = 2^t * u_t (host pre-scales X_t = 2^t * x_t;
   powers of two, so the recurrence stays bitwise identical), the decay
   TAU*... disappears and the update becomes a pure elementwise add:
       w_t = v_t * (v_t <= 2^t)     (DVE scalar_tensor_tensor)
       v_{t+1} = w_t + X_{t+1}      (plain tensor_tensor ADD)
       o_t = (v_t > 2^t)            (ACT sign pass)

3. DVE+GPSIMD split: GPSIMD runs tensor_tensor at ~1/2 DVE rate (8 Q7
   DSP cores, ~2.2 cyc/elem), so each update add is split ~1/3 DVE +
   ~2/3 GPSIMD (two chunks), and the DVE mask is issued in three
   matching sub-instructions so the cross-engine dependence pipelines.
   Per-step critical path drops from 8.9us (two DVE passes) to ~6.4us.

Rings: input loads on the sync HWDGE ring, spike stores on the scalar
HWDGE ring (issued in ACT program order), GPSIMD does pure compute.
"""

import numpy as np

import concourse.tile as tile
from concourse import bacc, mybir
from concourse.bass_utils import run_bass_kernel_spmd

T = 8
BS = 32
C = 128
HW = 32 * 32
NCORES = 8
BSH = BS // NCORES          # 4 batch elements per core
P = 128                     # SBUF partitions
FREE = BSH * C * HW // P    # 4096 f32 per partition per timestep
CHF = 2048                  # load-ramp chunk (half timestep)
DA = 1408                   # DVE's slice of each update add
G1 = (FREE - DA) // 2       # each of GPSIMD's two slices (1344)
VTH = 1.0
F32 = mybir.dt.float32
U8 = mybir.dt.uint8

_nc_cache = None


def _build():
    nc = bacc.Bacc("TRN2", target_bir_lowering=False, debug=False, num_devices=NCORES)
    x_d = nc.dram_tensor("x", [T, P, FREE], F32, kind="ExternalInput").ap()
    o_d = nc.dram_tensor("o", [T, P, FREE], U8, kind="ExternalOutput").ap()

    ADD = mybir.AluOpType.add
    ISLE = mybir.AluOpType.is_le
    MULT = mybir.AluOpType.mult
    SIGN = mybir.ActivationFunctionType.Sign

    with tile.TileContext(nc) as tc:
        with (
            tc.tile_pool(name="xa", bufs=1) as xa,
            tc.tile_pool(name="pp", bufs=2) as pp,
            tc.tile_pool(name="op", bufs=2) as op,
            tc.tile_pool(name="cp", bufs=1) as cp,
        ):
            # Per-partition bias constants -2^t for the ACT sign passes.
            nbias = cp.tile([P, T], F32)
            for t in range(T):
                nc.vector.memset(nbias[:, t:t + 1], -float(2 ** t))

            xt = xa.tile([P, T * FREE], F32)
            xv = x_d.rearrange("t p f -> p t f")  # [128, T, FREE] HBM view

            # Ramped loads (units of CHF=2048): small first so compute
            # starts early, large later so the ring stays short.
            load_ranges = [(0, 1), (1, 2), (2, 4), (4, 8), (8, 12), (12, 16)]
            for a, b in load_ranges:
                t0, f0 = divmod(a * CHF, FREE)
                t1, f1 = divmod(b * CHF, FREE)
                if f0 == 0 and f1 == 0:
                    src = xv[:, t0:t1, :]
                else:
                    assert t1 == t0 and f1 > f0 or (t1 == t0 + 1 and f1 == 0)
                    src = xv[:, t0, f0:f1 if f1 else FREE]
                nc.sync.dma_start(out=xt[:, a * CHF:b * CHF], in_=src)

            # Free-dim split points for the update/mask/sign/store chain.
            cuts = [(0, DA), (DA, DA + G1), (DA + G1, FREE)]

            w_prev = None
            for t in range(T):
                vs = xt[:, t * FREE:(t + 1) * FREE]  # v_t (in place over X_t)
                o = op.tile([P, FREE], U8, name="o", tag="o")
                wn = (
                    pp.tile([P, FREE], F32, name="w", tag="w")
                    if t < T - 1 else None
                )
                theta = float(2 ** t)
                if t == 0:
                    # v_0 = X_0 directly; halves for an early start.
                    spans = [(0, CHF), (CHF, FREE)]
                else:
                    spans = cuts
                for i, (lo, hi) in enumerate(spans):
                    if t > 0:
                        # v_t = w_{t-1} + X_t (in place); DVE takes the
                        # first slice, GPSIMD the other two.
                        eng = nc.vector if i == 0 else nc.gpsimd
                        eng.tensor_tensor(
                            vs[:, lo:hi], w_prev[:, lo:hi], vs[:, lo:hi],
                            op=ADD,
                        )
                    # o_t = sign(v_t - 2^t) as uint8: spike iff byte == 1.
                    nc.scalar.activation(
                        o[:, lo:hi], vs[:, lo:hi], SIGN,
                        bias=nbias[:, t:t + 1], scale=1.0,
                    )
                    if wn is not None:
                        # w_t = (v_t <= 2^t) * v_t
                        nc.vector.scalar_tensor_tensor(
                            wn[:, lo:hi], vs[:, lo:hi], theta, vs[:, lo:hi],
                            op0=ISLE, op1=MULT,
                        )
                    # Spike store on the scalar HWDGE ring, right after the
                    # sign pass in ACT program order.
                    if t == T - 1:
                        q = (hi - lo) // 2
                        nc.scalar.dma_start(
                            out=o_d[t][:, lo:lo + q], in_=o[:, lo:lo + q])
                        nc.scalar.dma_start(
                            out=o_d[t][:, lo + q:hi], in_=o[:, lo + q:hi])
                    else:
                        nc.scalar.dma_start(out=o_d[t][:, lo:hi], in_=o[:, lo:hi])
                w_prev = wn

    nc.compile()
    return nc


def _get_nc():
    global _nc_cache
    if _nc_cache is None:
        _nc_cache = _build()
    return _nc_cache


def _run(x: np.ndarray, **spmd_kwargs):
    nc = _get_nc()
    xr = np.asarray(x, dtype=np.float32).reshape(T, BS, C, HW)
    # Pre-scale X_t = 2^t * x_t (exact power-of-two scaling).
    scale = (2.0 ** np.arange(T, dtype=np.float32)).reshape(T, 1, 1, 1)
    xs = xr * scale
    in_maps = [
        {"x": np.ascontiguousarray(xs[:, k * BSH:(k + 1) * BSH]).reshape(T, P, FREE)}
        for k in range(NCORES)
    ]
    res = run_bass_kernel_spmd(nc, in_maps, core_ids=list(range(NCORES)), **spmd_kwargs)
    out = np.empty((T, BS, C, HW), dtype=np.float32)
    for k in range(NCORES):
        ok = res.results[k]["o"].reshape(T, BSH, C, HW)
        out[:, k * BSH:(k + 1) * BSH] = (ok == 1)
    return out.reshape(T * BS, C, 32, 32), res


def kernel(x: np.ndarray) -> np.ndarray:
    out, _ = _run(x)
    return out


# revision 9
# speedup vs baseline: 1.0231x; 1.0231x over previous
"""LIF (leaky integrate-and-fire) spiking recurrence on 8 Trainium2 cores.

Full input x: [T*bs, C, H, W] = [256, 128, 32, 32] f32 with T=8, bs=32.
Recurrence over T only, elementwise elsewhere:
    u_t = TAU * u_{t-1} * (1 - (u_{t-1} > VTH)) + x_t ;  o_t = (u_t > VTH)

Sharding: fully data-parallel over batch (bs=32 -> 4 per core), no
collectives. Each core's [4,128,32,32] per-timestep slab is a flat
[128 partitions, 4096] tile; the whole 16 MiB input stays SBUF-resident
and the state is computed in place over it.

Three structural tricks vs the naive f32 version (~97us):

1. Byte spikes: output is pure 0/1, so ACT computes s = sign(v - 2^t) as
   uint8 in one pass; host maps byte==1 -> 1.0f (exact under any
   float->u8 conversion semantics). Store traffic 16.78 -> 4.19 MB/core.

2. Scaled state: with v_t = 2^t * u_t (host pre-scales X_t = 2^t * x_t;
   powers of two, so the recurrence stays bitwise identical), the decay
   TAU*... disappears and the update becomes a pure elementwise add:
       w_t = v_t * (v_t <= 2^t)     (DVE scalar_tensor_tensor)
       v_{t+1} = w_t + X_{t+1}      (plain tensor_tensor ADD)
       o_t = (v_t > 2^t)            (ACT sign pass)

3. DVE+GPSIMD split: GPSIMD runs tensor_tensor at ~1/2 DVE rate (8 Q7
   DSP cores, ~2.2 cyc/elem), so each update add is split ~1/3 DVE +
   ~2/3 GPSIMD (two chunks), and the DVE mask is issued in three
   matching sub-instructions so the cross-engine dependence pipelines.
   Per-step critical path drops from 8.9us (two DVE passes) to ~6.4us.

Rings: input loads on the sync HWDGE ring, spike stores on the scalar
HWDGE ring (issued in ACT program order), GPSIMD does pure compute.
"""

import numpy as np

import concourse.tile as tile
from concourse import bacc, mybir
from concourse.bass_utils import run_bass_kernel_spmd

T = 8
BS = 32
C = 128
HW = 32 * 32
NCORES = 8
BSH = BS // NCORES          # 4 batch elements per core
P = 128                     # SBUF partitions
FREE = BSH * C * HW // P    # 4096 f32 per partition per timestep
CHF = 2048                  # load-ramp chunk (half timestep)
DA = 1408                   # DVE's slice of each update add
G1 = 1280                   # GPSIMD's first slice (cuts at multiples of 128:
                            # off-multiple free sizes drop DVE to ~1/3 rate)
VTH = 1.0
F32 = mybir.dt.float32
U8 = mybir.dt.uint8

_nc_cache = None


def _build():
    nc = bacc.Bacc("TRN2", target_bir_lowering=False, debug=False, num_devices=NCORES)
    x_d = nc.dram_tensor("x", [T, P, FREE], F32, kind="ExternalInput").ap()
    o_d = nc.dram_tensor("o", [T, P, FREE], U8, kind="ExternalOutput").ap()

    ADD = mybir.AluOpType.add
    ISLE = mybir.AluOpType.is_le
    MULT = mybir.AluOpType.mult
    SIGN = mybir.ActivationFunctionType.Sign

    with tile.TileContext(nc) as tc:
        with (
            tc.tile_pool(name="xa", bufs=1) as xa,
            tc.tile_pool(name="pp", bufs=2) as pp,
            tc.tile_pool(name="op", bufs=2) as op,
            tc.tile_pool(name="cp", bufs=1) as cp,
        ):
            # Per-partition bias constants -2^t for the ACT sign passes.
            nbias = cp.tile([P, T], F32)
            for t in range(T):
                nc.vector.memset(nbias[:, t:t + 1], -float(2 ** t))

            xt = xa.tile([P, T * FREE], F32)
            xv = x_d.rearrange("t p f -> p t f")  # [128, T, FREE] HBM view

            # Ramped loads (units of CHF=2048): small first so compute
            # starts early, large later so the ring stays short.
            load_ranges = [(0, 1), (1, 2), (2, 4), (4, 8), (8, 12), (12, 16)]
            for a, b in load_ranges:
                t0, f0 = divmod(a * CHF, FREE)
                t1, f1 = divmod(b * CHF, FREE)
                if f0 == 0 and f1 == 0:
                    src = xv[:, t0:t1, :]
                else:
                    assert t1 == t0 and f1 > f0 or (t1 == t0 + 1 and f1 == 0)
                    src = xv[:, t0, f0:f1 if f1 else FREE]
                nc.sync.dma_start(out=xt[:, a * CHF:b * CHF], in_=src)

            # Free-dim split points for the update/mask/sign/store chain.
            cuts = [(0, DA), (DA, DA + G1), (DA + G1, FREE)]

            w_prev = None
            for t in range(T):
                vs = xt[:, t * FREE:(t + 1) * FREE]  # v_t (in place over X_t)
                o = op.tile([P, FREE], U8, name="o", tag="o")
                wn = (
                    pp.tile([P, FREE], F32, name="w", tag="w")
                    if t < T - 1 else None
                )
                theta = float(2 ** t)
                if t == 0:
                    # v_0 = X_0 directly; halves for an early start.
                    spans = [(0, CHF), (CHF, FREE)]
                else:
                    spans = cuts
                for i, (lo, hi) in enumerate(spans):
                    if t > 0:
                        # v_t = w_{t-1} + X_t (in place); DVE takes the
                        # first slice, GPSIMD the other two.
                        eng = nc.vector if i == 0 else nc.gpsimd
                        eng.tensor_tensor(
                            vs[:, lo:hi], w_prev[:, lo:hi], vs[:, lo:hi],
                            op=ADD,
                        )
                    # o_t = sign(v_t - 2^t) as uint8: spike iff byte == 1.
                    nc.scalar.activation(
                        o[:, lo:hi], vs[:, lo:hi], SIGN,
                        bias=nbias[:, t:t + 1], scale=1.0,
                    )
                    if wn is not None:
                        # w_t = (v_t <= 2^t) * v_t
                        nc.vector.scalar_tensor_tensor(
                            wn[:, lo:hi], vs[:, lo:hi], theta, vs[:, lo:hi],
                            op0=ISLE, op1=MULT,
                        )
                    # Spike store on the scalar HWDGE ring, right after the
                    # sign pass in ACT program order.
                    if t == T - 1:
                        q = (hi - lo) // 2
                        nc.scalar.dma_start(
                            out=o_d[t][:, lo:lo + q], in_=o[:, lo:lo + q])
                        nc.scalar.dma_start(
                            out=o_d[t][:, lo + q:hi], in_=o[:, lo + q:hi])
                    else:
                        nc.scalar.dma_start(out=o_d[t][:, lo:hi], in_=o[:, lo:hi])
                w_prev = wn

    nc.compile()
    return nc


def _get_nc():
    global _nc_cache
    if _nc_cache is None:
        _nc_cache = _build()
    return _nc_cache


def _run(x: np.ndarray, **spmd_kwargs):
    nc = _get_nc()
    xr = np.asarray(x, dtype=np.float32).reshape(T, BS, C, HW)
    # Pre-scale X_t = 2^t * x_t (exact power-of-two scaling).
    scale = (2.0 ** np.arange(T, dtype=np.float32)).reshape(T, 1, 1, 1)
    xs = xr * scale
    in_maps = [
        {"x": np.ascontiguousarray(xs[:, k * BSH:(k + 1) * BSH]).reshape(T, P, FREE)}
        for k in range(NCORES)
    ]
    res = run_bass_kernel_spmd(nc, in_maps, core_ids=list(range(NCORES)), **spmd_kwargs)
    out = np.empty((T, BS, C, HW), dtype=np.float32)
    for k in range(NCORES):
        ok = res.results[k]["o"].reshape(T, BSH, C, HW)
        out[:, k * BSH:(k + 1) * BSH] = (ok == 1)
    return out.reshape(T * BS, C, 32, 32), res


def kernel(x: np.ndarray) -> np.ndarray:
    out, _ = _run(x)
    return out
